# revision 18
# baseline (speedup 1.0000x reference)
"""Multi-headed attention on 8 Trainium2 NeuronCores.

Problem: B=2, S=2048, D=1024, H=16 heads (dph=64), boolean attention mask.
    y = softmax(mask_fill((XqWq+bq)(XkWk+bk)^T / 8)) (XvWv+bv) Wo + bo

Sharding (Megatron-style, data + tensor parallel):
  core c = 4*b + g  handles batch b (2-way DP) and head group g (4 heads,
  256 head-dims, 4-way TP).  Wq/Wk/Wv column-sharded, Wo row-sharded.
  Each core emits a partial y for its batch; the host sums the 4 partials
  per batch and adds bo (row-parallel reduction done on host).

Device kernel layout choices:
  - Host pre-transposes/casts inputs to fp16: Xq^T/Xk^T/Xv^T [D,S] so the
    contraction dim lands on SBUF partitions with zero on-device transposes.
  - Scores are computed transposed, S^T[k,q] (k on partitions), via
    lhsT=K^T chunk [64,128], rhs=Q^T [64,512].  Heads alternate base
    partitions 0/64 so pairs of K=64 matmuls pack into the PE array.
  - Mask is applied multiplicatively after exp (exact: exp(-inf)=0), using
    a host-prepared (1-mask)^T fp16 tensor; softmax max-subtraction is
    skipped (scores ~ N(0,1), exp can't overflow; same math as reference).
  - V is augmented with a leading ones column, so the attention*V matmul
    also produces the softmax denominators (row 0 of ctx^T psum) for free.
  - Normalization (1/sums) happens after the AV matmul; reciprocal on DVE,
    broadcast across partitions via gpsimd partition_broadcast (DMA).
  - Output projection consumes ctx^T directly as lhsT (no transposes).
"""

import numpy as np

import concourse.bass as bass
import concourse.mybir as mybir
import concourse.tile as tile
from concourse import bacc
from concourse.bass_utils import run_bass_kernel_spmd

B, S, D, H = 2, 2048, 1024, 16
DPH = 64
NCORES = 8
HG = 4                 # heads per core
DHC = HG * DPH         # head dims per core = 256
NDC = D // 128         # contraction chunks for projections = 8
NKB = S // 128         # key blocks = 16
F16 = mybir.dt.float16
F32 = mybir.dt.float32
EXP = mybir.ActivationFunctionType.Exp
COPY = mybir.ActivationFunctionType.Copy

_PROGRAM = None


def _emit(tc):
    nc = tc.nc

    # --- DRAM I/O (per core) ---
    xq_t = nc.declare_dram_parameter("xq_t", [D, S], F16, isOutput=False).ap()
    xk_t = nc.declare_dram_parameter("xk_t", [D, S], F16, isOutput=False).ap()
    xv_t = nc.declare_dram_parameter("xv_t", [D, S], F16, isOutput=False).ap()
    m01_t = nc.declare_dram_parameter("m01_t", [S, S], F16, isOutput=False).ap()
    wq = nc.declare_dram_parameter("wq", [D, DHC], F16, isOutput=False).ap()
    wk = nc.declare_dram_parameter("wk", [D, DHC], F16, isOutput=False).ap()
    wv = nc.declare_dram_parameter("wv", [D, DHC], F16, isOutput=False).ap()
    wo = nc.declare_dram_parameter("wo", [DHC, D], F16, isOutput=False).ap()
    bq = nc.declare_dram_parameter("bq", [1, DHC], F16, isOutput=False).ap()
    bk = nc.declare_dram_parameter("bk", [1, DHC], F16, isOutput=False).ap()
    bv = nc.declare_dram_parameter("bv", [1, DHC], F16, isOutput=False).ap()
    y = nc.declare_dram_parameter("y", [S, D], F32, isOutput=True).ap()

    from contextlib import ExitStack

    with ExitStack() as ctx:
        # Persistent SBUF tensors (one slot each: distinct tags).
        wp = ctx.enter_context(tc.tile_pool(name="wts", bufs=1))
        # Streaming pools.
        big = ctx.enter_context(tc.tile_pool(name="big", bufs=3))
        ep = ctx.enter_context(tc.tile_pool(name="e", bufs=3))
        e2p = ctx.enter_context(tc.tile_pool(name="e2", bufs=4))
        cup = ctx.enter_context(tc.tile_pool(name="cu", bufs=2))
        drp = ctx.enter_context(tc.tile_pool(name="dr", bufs=3))
        yp = ctx.enter_context(tc.tile_pool(name="y", bufs=2))
        psp = ctx.enter_context(tc.tile_pool(name="ps", bufs=2, space="PSUM"))
        pcp = ctx.enter_context(tc.tile_pool(name="pc", bufs=2, space="PSUM"))

        # --- load weights/biases ---
        wq_s = wp.tile([128, NDC, DHC], F16, tag="wq")
        wk_s = wp.tile([128, NDC, DHC], F16, tag="wk")
        wv_s = wp.tile([128, NDC, DHC], F16, tag="wv")
        wo_s = wp.tile([128, 2, D], F16, tag="wo")
        bq_s = wp.tile([1, DHC], F16, tag="bq")
        bk_s = wp.tile([1, DHC], F16, tag="bk")
        bv_s = wp.tile([1, DHC], F16, tag="bv")
        ones_s = wp.tile([1, 512], F16, tag="ones")
        ones64_s = wp.tile([65, 64], F32, tag="ones64")
        kt = wp.tile([128, 2, S], F16, tag="kt")
        qt = wp.tile([128, 2, S], F16, tag="qt")
        vsb = wp.tile([128, NKB, HG * 65], F16, tag="vsb")
        ctxn = wp.tile([128, 2, S], F16, tag="ctxn")

        nc.sync.dma_start(out=wq_s[:], in_=wq.rearrange("(c p) m -> p c m", p=128))
        nc.sync.dma_start(out=wk_s[:], in_=wk.rearrange("(c p) m -> p c m", p=128))
        nc.sync.dma_start(out=wv_s[:], in_=wv.rearrange("(c p) m -> p c m", p=128))
        nc.sync.dma_start(out=wo_s[:], in_=wo.rearrange("(j p) d -> p j d", p=128))
        nc.sync.dma_start(out=bq_s[:], in_=bq[:])
        nc.sync.dma_start(out=bk_s[:], in_=bk[:])
        nc.sync.dma_start(out=bv_s[:], in_=bv[:])
        nc.vector.memset(ones_s[:], 1.0)
        nc.vector.memset(ones64_s[:], 1.0)
        # ones column (index 64 of each head's 65-wide group) in V_aug
        for h in range(HG):
            nc.vector.memset(vsb[:, :, h * 65 + 64 : h * 65 + 65], 1.0)

        # --- load activations (transposed), projections ---
        def load_xt(src):
            t = big.tile([128, NDC, S], F16, tag="big")
            nc.sync.dma_start(out=t[:], in_=src.rearrange("(c p) s -> p c s", p=128))
            return t

        def proj_qk(xt, w_s, b_s, out_sb):
            # out_sb[dh, s] = (X @ W + b)^T for this core's 256 head dims
            for j in range(2):
                for t in range(4):
                    ps = psp.tile([128, 512], F32, tag="ps")
                    for c in range(NDC):
                        nc.tensor.matmul(
                            ps[:],
                            lhsT=w_s[:, c, j * 128 : (j + 1) * 128],
                            rhs=xt[:, c, t * 512 : (t + 1) * 512],
                            start=(c == 0),
                            stop=False,
                        )
                    nc.tensor.matmul(
                        ps[:],
                        lhsT=b_s[0:1, j * 128 : (j + 1) * 128],
                        rhs=ones_s[0:1, :],
                        start=False,
                        stop=True,
                    )
                    nc.vector.tensor_copy(out_sb[:, j, t * 512 : (t + 1) * 512], ps[:])

        xk_sb = load_xt(xk_t)
        xv_sb = load_xt(xv_t)
        proj_qk(xk_sb, wk_s, bk_s, kt)
        xq_sb = load_xt(xq_t)

        # V (natural layout [s, dh]) + bias via K=1 ones matmul
        for i in range(NKB):
            ps = psp.tile([128, DHC], F32, tag="ps")
            for c in range(NDC):
                nc.tensor.matmul(
                    ps[:],
                    lhsT=xv_sb[:, c, i * 128 : (i + 1) * 128],
                    rhs=wv_s[:, c, :],
                    start=(c == 0),
                    stop=False,
                )
            nc.tensor.matmul(
                ps[:], lhsT=ones_s[0:1, 0:128], rhs=bv_s[0:1, :], start=False, stop=True
            )
            nc.vector.tensor_copy(
                vsb[:, i, :].rearrange("p (h e) -> p h e", e=65)[:, :, 0:64],
                ps.rearrange("p (h d) -> p h d", d=DPH),
            )

        proj_qk(xq_sb, wq_s, bq_s, qt)

        # --- attention ---
        def drain_head(pc, h, qh):
            # pc: [65, 1024] psum: rows 0..63 = ctx^T (unnormalized), row 64 = sums
            cu_t = cup.tile([65, 1024], F32, tag="cu")
            nc.vector.tensor_copy(cu_t[:], pc[:])
            ri = drp.tile([65, 1024], F32, tag="ri")
            nc.vector.reciprocal(ri[64:65, :], cu_t[64:65, :])
            # broadcast 1/sums across 64 partitions via K=1 ones matmul
            rbc = psp.tile([128, 1024], F32, tag="ps", name=f"rbc_{h}_{qh}")
            for dq in range(2):
                nc.tensor.matmul(
                    rbc[0:64, dq * 512 : (dq + 1) * 512],
                    lhsT=ones64_s[64:65, :],
                    rhs=ri[64:65, dq * 512 : (dq + 1) * 512],
                    start=True,
                    stop=True,
                )
            ct = drp.tile([64, 1024], F16, tag="ct")
            nc.vector.tensor_mul(ct[0:64, :], cu_t[0:64, :], rbc[0:64, :])
            nc.sync.dma_start(
                out=ctxn[(h % 2) * 64 : (h % 2) * 64 + 64, h // 2,
                         qh * 1024 : (qh + 1) * 1024],
                in_=ct[0:64, :],
            )

        for qh in range(2):
            msk = big.tile([128, NKB, 1024], F16, tag="big")
            nc.sync.dma_start(
                out=msk[:],
                in_=m01_t.rearrange("(c p) q -> p c q", p=128)[
                    :, :, qh * 1024 : (qh + 1) * 1024
                ],
            )
            for hp in range(2):
                pcs = [
                    pcp.tile([65, 1024], F32, tag="pc", name=f"pc_{qh}_{hp}_{i}")
                    for i in range(2)
                ]
                for kc in range(NKB):
                    for hh in range(2):
                        h = 2 * hp + hh
                        base = hh * 64
                        ps = psp.tile([128, 1024], F32, tag="ps")
                        for dq in range(2):
                            nc.tensor.matmul(
                                ps[:, dq * 512 : (dq + 1) * 512],
                                lhsT=kt[base : base + 64, hp, kc * 128 : (kc + 1) * 128],
                                rhs=qt[base : base + 64, hp,
                                       qh * 1024 + dq * 512 : qh * 1024 + (dq + 1) * 512],
                                start=True,
                                stop=True,
                            )
                        e = ep.tile([128, 1024], F16, tag="e")
                        nc.scalar.activation(e[:], ps[:], EXP)
                        e2 = e2p.tile([128, 1024], F16, tag="e2")
                        nc.vector.tensor_mul(e2[:], e[:], msk[:, kc, :])
                        for dq in range(2):
                            nc.tensor.matmul(
                                pcs[hh][:, dq * 512 : (dq + 1) * 512],
                                lhsT=vsb[:, kc, h * 65 : (h + 1) * 65],
                                rhs=e2[:, dq * 512 : (dq + 1) * 512],
                                start=(kc == 0),
                                stop=(kc == NKB - 1),
                            )
                for hh in range(2):
                    drain_head(pcs[hh], 2 * hp + hh, qh)

        # --- output projection: y[s, d] = ctx @ Wo (partial over this core's dh) ---
        for sb in range(NKB):
            yt = yp.tile([128, D], F32, tag="yt")
            for dt in range(2):
                ps = psp.tile([128, 512], F32, tag="ps")
                for j in range(2):
                    nc.tensor.matmul(
                        ps[:],
                        lhsT=ctxn[:, j, sb * 128 : (sb + 1) * 128],
                        rhs=wo_s[:, j, dt * 512 : (dt + 1) * 512],
                        start=(j == 0),
                        stop=(j == 1),
                    )
                if dt == 0:
                    nc.vector.tensor_copy(yt[:, 0:512], ps[:])
                else:
                    nc.scalar.activation(yt[:, 512:1024], ps[:], COPY)
            nc.sync.dma_start(out=y[sb * 128 : (sb + 1) * 128, :], in_=yt[:])


def _get_program():
    global _PROGRAM
    if _PROGRAM is None:
        nc = bacc.Bacc("TRN2", target_bir_lowering=False, debug=False)
        with tile.TileContext(nc) as tc:
            _emit(tc)
        nc.compile()
        _PROGRAM = nc
    return _PROGRAM


def _make_in_maps(key, value, query, mask, Wq, bq, Wk, bk, Wv, bv, Wo, bo):
    key = np.asarray(key, np.float32)
    value = np.asarray(value, np.float32)
    query = np.asarray(query, np.float32)
    mask = np.asarray(mask, bool)
    Wq = np.asarray(Wq, np.float32)
    Wk = np.asarray(Wk, np.float32)
    Wv = np.asarray(Wv, np.float32)
    Wo = np.asarray(Wo, np.float32)
    bq = np.asarray(bq, np.float32)
    bk = np.asarray(bk, np.float32)
    bv = np.asarray(bv, np.float32)

    per_batch = []
    for b in range(B):
        per_batch.append(
            dict(
                xq_t=np.ascontiguousarray(query[b].T.astype(np.float16)),
                xk_t=np.ascontiguousarray(key[b].T.astype(np.float16)),
                xv_t=np.ascontiguousarray(value[b].T.astype(np.float16)),
                m01_t=np.ascontiguousarray((~mask[b]).T.astype(np.float16)),
            )
        )
    in_maps = []
    for c in range(NCORES):
        b, g = divmod(c, HG)
        gs, ge = g * DHC, (g + 1) * DHC
        in_maps.append(
            dict(
                per_batch[b],
                wq=np.ascontiguousarray((Wq[:, gs:ge] / 8.0).astype(np.float16)),
                wk=np.ascontiguousarray(Wk[:, gs:ge].astype(np.float16)),
                wv=np.ascontiguousarray(Wv[:, gs:ge].astype(np.float16)),
                wo=np.ascontiguousarray(Wo[gs:ge, :].astype(np.float16)),
                bq=np.ascontiguousarray((bq[gs:ge] / 8.0).astype(np.float16).reshape(1, DHC)),
                bk=np.ascontiguousarray(bk[gs:ge].astype(np.float16).reshape(1, DHC)),
                bv=np.ascontiguousarray(bv[gs:ge].astype(np.float16).reshape(1, DHC)),
            )
        )
    return in_maps


def _run(in_maps, trace=False, **kw):
    nc = _get_program()
    return run_bass_kernel_spmd(nc, in_maps, list(range(NCORES)), trace=trace, **kw)


def kernel(key, value, query, mask, Wq, bq, Wk, bk, Wv, bv, Wo, bo):
    in_maps = _make_in_maps(key, value, query, mask, Wq, bq, Wk, bk, Wv, bv, Wo, bo)
    res = _run(in_maps).results
    bo = np.asarray(bo, np.float32)
    y = np.zeros((B, S, D), np.float32)
    for c in range(NCORES):
        y[c // HG] += res[c]["y"]
    y += bo[None, None, :]
    return y
